# revision 30
# baseline (speedup 1.0000x reference)
"""Trainium2 Bass kernel for nn_Attention (LayerNorm -> MHA -> out-proj).

Full (unsharded) inputs in, full output out. Shards across 8 NeuronCores as
(batch b in 0..3) x (head-group g in 0..1): core c = 2*b + g computes batch b,
heads [g*8, g*8+8), producing a partial projection [2048, 1024] (fp16); the
host sums the two group partials per batch and adds b_out.

Device program (per core), designed so ScalarE does (almost) nothing but the
33.5M softmax exps -- the critical path -- while PE/DVE/GPSIMD hide under it:

  1. x arrives host-pre-transposed: xT [1024(dim), 2048(tok)] fp16. LayerNorm
     stats via PE: sum(x) and sum(x^2) as ones-column matmuls (token sums land
     in the free dim); mu/var/rstd derived with DVE row ops + ACT Ln/Exp
     (rstd = exp(-0.5*ln(var+eps)); same activation table set as softmax Exp).
  2. QKV on raw xT: Q^T/K^T [512, 2048] (2 heads per 128-row tile) and
     V [tok, 8*(64+1)] (65th col = 1.0 -> PV also yields the softmax denom).
     The -mu correction is a rank-1 matmul accumulated into each PSUM tile
     (lhsT = row-sums of W, rhs = -mu row); rstd (and the softmax scale,
     host-folded into Wq) is applied by the DVE PSUM->SBUF evacuation
     (tensor_tensor multiply with a broadcast rstd tile -- same cost as the
     copy it replaces).
  3. Attention per head-pair p: S^T pair-packed via PE row-tiling (contraction
     is only 64, so head 2p runs in array rows 0-63 concurrently with head
     2p+1 in rows 64-127); one exp per (kb, qq) over both heads' scores
     [128, 1024]; PV with M=65 (fused denominator). Normalize: DVE reciprocal
     of the denom row, GPSIMD partition-broadcast, DVE multiply -> outT fp16.
  4. Projection outT.T @ wo per token tile, fp16 out, summed on host.

  Phase overlap: V tiles are computed just-in-time inside the first attention
  window; Q/K for pair p+1 are emitted during pair p's attention; projection
  interleaves with the last pair. PSUM budget: sps 2x[128,1024] (4 banks) +
  pvA/pvB [128,512]x2 (2) + qkv/stats/proj ring (2) = 8.
"""

import sys

if "/opt/trn_rl_repo" not in sys.path:
    sys.path.insert(0, "/opt/trn_rl_repo")

import numpy as np

import concourse.tile as tile
from concourse import bacc, mybir
from concourse.bass_utils import run_bass_kernel_spmd
from concourse.masks import make_identity

P = 128
N_TOK = 2048
DIM = 1024
HEADS_TOTAL = 16
H = 8  # heads per core
DH = 64
GI = H * DH  # 512, per-core inner size
INNER = HEADS_TOTAL * DH  # 1024
N_CORES = 8
SCALE = DH ** -0.5
EPS = 1e-5
NDT = DIM // P  # 8 d-tiles
NQB = N_TOK // 512  # 4 token quarters
NTT = N_TOK // P  # 16 token tiles

AF = mybir.ActivationFunctionType
ALU = mybir.AluOpType
f32 = mybir.dt.float32
fp16 = mybir.dt.float16

_CACHE = {}


def build_nc(apply_b=False):
    nc = bacc.Bacc("TRN2", target_bir_lowering=False, debug=False)
    xt_d = nc.dram_tensor("xt", [DIM, N_TOK], fp16, kind="ExternalInput").ap()
    wq_d = nc.dram_tensor("wq", [P, NDT, GI], fp16, kind="ExternalInput").ap()
    wk_d = nc.dram_tensor("wk", [P, NDT, GI], fp16, kind="ExternalInput").ap()
    wv_d = nc.dram_tensor("wv", [P, NDT, GI], fp16, kind="ExternalInput").ap()
    wo_d = nc.dram_tensor("wo", [4, P, DIM], fp16, kind="ExternalInput").ap()
    ws_d = nc.dram_tensor("ws", [1, 3, GI], fp16, kind="ExternalInput").ap()
    wsc_d = nc.dram_tensor("wsc", [P, 3, 4], f32, kind="ExternalInput").ap()
    wb_d = wbc_d = None
    if apply_b:
        wb_d = nc.dram_tensor("wb", [1, 3, GI], fp16,
                              kind="ExternalInput").ap()
        wbc_d = nc.dram_tensor("wbc", [P, 2, 4], f32,
                               kind="ExternalInput").ap()
    out_d = nc.dram_tensor("out", [N_TOK, DIM], fp16, kind="ExternalOutput").ap()
    scr_d = nc.dram_tensor("scr", [1, N_TOK], fp16).ap()  # rstd row->col bounce

    with tile.TileContext(nc) as tc:
        _body(nc, tc, xt_d, wq_d, wk_d, wv_d, wo_d, ws_d, wsc_d, wb_d,
              wbc_d, out_d)
    nc.compile()
    return nc


def _body(nc, tc, xt_d, wq_d, wk_d, wv_d, wo_d, ws_d, wsc_d, wb_d, wbc_d,
          out_d):
    apply_b = wb_d is not None
    # ---- persistent SBUF ----
    xT = nc.alloc_sbuf_tensor("xT", [P, NDT, N_TOK], fp16)
    QT = [nc.alloc_sbuf_tensor(f"qt{p}", [P, N_TOK], fp16) for p in range(4)]
    KT = [nc.alloc_sbuf_tensor(f"kt{p}", [P, N_TOK], fp16) for p in range(4)]
    V = nc.alloc_sbuf_tensor("vt", [P, NTT, H, DH + 1], fp16)
    outT = [nc.alloc_sbuf_tensor(f"ot{p}", [P, N_TOK], fp16) for p in range(4)]
    wq_sb = nc.alloc_sbuf_tensor("wqs", [P, NDT, GI], fp16)
    wk_sb = nc.alloc_sbuf_tensor("wks", [P, NDT, GI], fp16)
    wv_sb = nc.alloc_sbuf_tensor("wvs", [P, NDT, GI], fp16)
    wo_sb = [nc.alloc_sbuf_tensor(f"wos{p}", [P, DIM], fp16) for p in range(4)]
    ws_sb = nc.alloc_sbuf_tensor("wss", [1, 3, GI], fp16)  # wsumq/k/v rows
    wsc_sb = nc.alloc_sbuf_tensor("wscs", [P, 3, 4], f32)  # wsum f-chunk cols
    wb_sb = nc.alloc_sbuf_tensor("wbs", [1, 3, GI], fp16) if apply_b else None
    wbc_sb = nc.alloc_sbuf_tensor("wbcs", [P, 2, 4], f32) if apply_b else None
    # small fp16 rows (partition 0): -mu, E[x^2]+eps, scratch, rstd, -mu*rstd
    nmu16 = nc.alloc_sbuf_tensor("nmu16", [1, N_TOK], fp16)
    m2row = nc.alloc_sbuf_tensor("m2row", [1, N_TOK], fp16)
    tmprow = nc.alloc_sbuf_tensor("tmprow", [1, N_TOK], fp16)
    rstd_row = nc.alloc_sbuf_tensor("rstd_row", [1, N_TOK], fp16)
    mr_row = nc.alloc_sbuf_tensor("mr_row", [1, N_TOK], fp16)
    rstd_bc = nc.alloc_sbuf_tensor("rstd_bc", [P, N_TOK], fp16)
    mr_bc = nc.alloc_sbuf_tensor("mr_bc", [P, N_TOK], fp16)
    rstd_col = nc.alloc_sbuf_tensor("rstd_col", [P, NTT], f32)
    mr_col = nc.alloc_sbuf_tensor("mr_col", [P, NTT], f32)
    wsv_bc = nc.alloc_sbuf_tensor("wsv_bc", [P, GI], fp16)
    bv_bc = nc.alloc_sbuf_tensor("bv_bc", [P, GI], fp16) if apply_b else None

    # V denominator ones-column (never overwritten by evacuations)
    nc.vector.memset(V[:, :, :, DH : DH + 1], 1.0)

    # ---- DMAs: x quarter 0 first (stats chain), weights, rest of x ----
    dmaq = [nc.sync, nc.scalar, nc.gpsimd]

    def dma_x(qb):
        cs = slice(qb * 512, (qb + 1) * 512)
        for dt in range(NDT):
            dmaq[dt % 3].dma_start(
                xT[:, dt, cs], xt_d[dt * P : (dt + 1) * P, cs])

    dma_x(0)
    nc.sync.dma_start(ws_sb[:, :, :], ws_d[:, :, :])
    nc.scalar.dma_start(wsc_sb[:, :, :], wsc_d[:, :, :])
    if apply_b:
        nc.sync.dma_start(wb_sb[:, :, :], wb_d[:, :, :])
        nc.scalar.dma_start(wbc_sb[:, :, :], wbc_d[:, :, :])
    nc.gpsimd.dma_start(wq_sb[:, :, :], wq_d[:, :, :])
    nc.sync.dma_start(wk_sb[:, :, :], wk_d[:, :, :])
    dma_x(1)
    nc.scalar.dma_start(wv_sb[:, :, :], wv_d[:, :, :])
    dma_x(2)
    dma_x(3)
    for p in range(4):
        dmaq[p % 3].dma_start(wo_sb[p][:, :], wo_d[p])

    # ---- pools ----
    with tc.tile_pool(name="sps", bufs=2, space="PSUM") as spool, \
         tc.tile_pool(name="pv", bufs=2, space="PSUM") as pvpool, \
         tc.tile_pool(name="qp", bufs=2, space="PSUM") as qpool, \
         tc.tile_pool(name="es", bufs=6) as espool, \
         tc.tile_pool(name="xq", bufs=1) as xqpool, \
         tc.tile_pool(name="bc", bufs=2) as bcpool, \
         tc.tile_pool(name="ob", bufs=3) as obpool:

        # ---- LN stats: sums over d via ones-matmuls ----
        ones16 = nc.alloc_sbuf_tensor("ones16", [P, 1], fp16)
        nc.vector.memset(ones16[:, :], 1.0)
        identf = nc.alloc_sbuf_tensor("identf", [P, P], f32)
        make_identity(nc, identf[:, :])
        identh = nc.alloc_sbuf_tensor("identh", [P, P], fp16)
        nc.vector.tensor_copy(identh[:, :], identf[:, :])
        def stats_chunk(qb):
            """LN stats + rstd for token quarter qb (all rows per-qb)."""
            cs = slice(qb * 512, (qb + 1) * 512)
            xsq = xqpool.tile([P, NDT, 512], fp16, tag="xq", name="xq")
            for dt in range(NDT):
                nc.vector.tensor_mul(xsq[:, dt, :], xT[:, dt, cs],
                                     xT[:, dt, cs])
            stx = qpool.tile([P, 512], f32, tag="qp", name="qp")
            for dt in range(NDT):
                nc.tensor.matmul(stx[0:1, :], ones16[:, :], xT[:, dt, cs],
                                 start=(dt == 0), stop=(dt == NDT - 1))
            nc.vector.tensor_scalar_mul(nmu16[0:1, cs], stx[0:1, :],
                                        -1.0 / DIM)
            stx2 = qpool.tile([P, 512], f32, tag="qp", name="qp")
            for dt in range(NDT):
                nc.tensor.matmul(stx2[0:1, :], ones16[:, :], xsq[:, dt, :],
                                 start=(dt == 0), stop=(dt == NDT - 1))
            nc.vector.tensor_scalar(m2row[0:1, cs], stx2[0:1, :],
                                    1.0 / DIM, EPS, op0=ALU.mult, op1=ALU.add)
            # var = m2 - mu^2; rstd = exp(-0.5*ln(var))
            nc.vector.tensor_mul(tmprow[0:1, cs], nmu16[0:1, cs],
                                 nmu16[0:1, cs])
            nc.vector.tensor_sub(m2row[0:1, cs], m2row[0:1, cs],
                                 tmprow[0:1, cs])
            nc.scalar.activation(tmprow[0:1, cs], m2row[0:1, cs], AF.Ln)
            nc.scalar.activation(rstd_row[0:1, cs], tmprow[0:1, cs],
                                 AF.Exp, scale=-0.5)
            nc.vector.tensor_mul(mr_row[0:1, cs], nmu16[0:1, cs],
                                 rstd_row[0:1, cs])
            nc.gpsimd.partition_broadcast(rstd_bc[:, cs], rstd_row[0:1, cs],
                                          channels=P)
            nc.gpsimd.partition_broadcast(mr_bc[:, cs], mr_row[0:1, cs],
                                          channels=P)
            # rstd/mr rows -> per-token-partition columns via PE transposes
            rcp = qpool.tile([P, 512], f32, tag="qp", name="qp")
            rc16 = rcp[:].bitcast(fp16).rearrange("p (t two) -> p t two",
                                                  two=2)
            for j in range(4):
                t = qb * 4 + j
                nc.tensor.transpose(rc16[:, j, 0:1],
                                    rstd_row[0:1, t * P : (t + 1) * P],
                                    identh[0:1, 0:1])
                nc.tensor.transpose(rc16[:, 4 + j, 0:1],
                                    mr_row[0:1, t * P : (t + 1) * P],
                                    identh[0:1, 0:1])
            nc.vector.tensor_copy(rstd_col[:, qb * 4 : qb * 4 + 4],
                                  rc16[:, 0:4, 0])
            nc.vector.tensor_copy(mr_col[:, qb * 4 : qb * 4 + 4],
                                  rc16[:, 4:8, 0])

        def qk_tile_units(dst, w_sb, wsi, pp, qb):
            """Closures (<=2 matmuls each) for one [128,512] Q^T/K^T tile."""
            cs = slice(qb * 512, (qb + 1) * 512)
            fs = slice(pp * P, (pp + 1) * P)
            cell = {}

            def dmms(d0):
                def f():
                    if d0 == 0:
                        cell["ps"] = qpool.tile([P, 512], f32, tag="qp",
                                                name="qp")
                    for dt in (d0, d0 + 1):
                        nc.tensor.matmul(cell["ps"][:, :], w_sb[:, dt, fs],
                                         xT[:, dt, cs],
                                         start=(dt == 0), stop=(dt == 7))
                return f

            def tail():
                # dst = psum*rstd + wsum_f * (-mu*rstd)  (+ beta_f)
                nc.vector.tensor_mul(dst[:, cs], cell["ps"][:, :],
                                     rstd_bc[:, cs])
                nc.vector.scalar_tensor_tensor(
                    dst[:, cs], mr_bc[:, cs], wsc_sb[:, wsi, pp : pp + 1],
                    dst[:, cs], op0=ALU.mult, op1=ALU.add)
                if apply_b:
                    nc.vector.tensor_scalar_add(
                        dst[:, cs], dst[:, cs],
                        wbc_sb[:, wsi, pp : pp + 1])

            return [dmms(0), dmms(2), dmms(4), dmms(6), tail]

        def v_tile(t):
            """One [128 tok, 512 feat] tile of V (token tile t)."""
            ts = slice(t * P, (t + 1) * P)
            ps = qpool.tile([P, 512], f32, tag="qp", name="qp")
            for dt in range(NDT):
                nc.tensor.matmul(ps[:, :], xT[:, dt, ts], wv_sb[:, dt, :],
                                 start=(dt == 0), stop=(dt == NDT - 1))
            vv = V[:, t, :, 0:DH]
            nc.vector.tensor_scalar(
                vv, ps[:].rearrange("p (h w) -> p h w", w=DH),
                rstd_col[:, t : t + 1], None, op0=ALU.mult)
            # V += wsumv_f * (-mu*rstd)_t  (+ beta_v)
            nc.vector.scalar_tensor_tensor(
                vv, wsv_bc[:, :].rearrange("p (h w) -> p h w", w=DH),
                mr_col[:, t : t + 1], vv, op0=ALU.mult, op1=ALU.add)
            if apply_b:
                nc.vector.tensor_add(
                    vv, vv, bv_bc[:, :].rearrange("p (h w) -> p h w", w=DH))

        def proj_units(t):
            """Closures for output projection of token tile t + DMA out."""
            ts = slice(t * P, (t + 1) * P)
            cell = {}

            def half(nn, first):
                def f():
                    if nn == 0 and first:
                        cell["ob"] = obpool.tile([P, DIM], fp16, tag="ob",
                                                 name="ob")
                    cs = slice(nn * 512, (nn + 1) * 512)
                    if first:
                        cell["pp"] = qpool.tile([P, 512], f32, tag="qp",
                                                name="qp")
                        for p in (0, 1):
                            nc.tensor.matmul(cell["pp"][:, :],
                                             outT[p][:, ts], wo_sb[p][:, cs],
                                             start=(p == 0), stop=False)
                    else:
                        for p in (2, 3):
                            nc.tensor.matmul(cell["pp"][:, :],
                                             outT[p][:, ts], wo_sb[p][:, cs],
                                             start=False, stop=(p == 3))
                        nc.vector.tensor_copy(cell["ob"][:, cs],
                                              cell["pp"][:, :])
                return f

            def dma():
                nc.sync.dma_start(out_d[ts, :], cell["ob"][:, :])

            return [half(0, True), half(0, False), half(1, True),
                    half(1, False), dma]

        bgq = []

        def att_block(p, qq, pvA, pvB, kb_range, bg):
            """S (pair-packed) -> exp -> PV for kb in kb_range."""
            hA, hB = 2 * p, 2 * p + 1
            cs = slice(qq * 512, (qq + 1) * 512)
            for kb in kb_range:
                ks = slice(kb * P, (kb + 1) * P)
                sps = spool.tile([P, 1024], f32, tag="sp", name="sp")
                nc.tensor.matmul(sps[:, 0:512], KT[p][0:DH, ks],
                                 QT[p][0:DH, cs], start=True, stop=True)
                nc.tensor.matmul(sps[:, 512:1024], KT[p][DH:P, ks],
                                 QT[p][DH:P, cs], start=True, stop=True)
                es = espool.tile([P, 1024], fp16, tag="es", name="es")
                nc.scalar.activation(es[:], sps[:], AF.Exp)
                if bg and bgq:
                    bgq.pop(0)()
                nc.tensor.matmul(pvA[0 : DH + 1, :], V[:, kb, hA, :],
                                 es[:, 0:512],
                                 start=(kb == 0), stop=(kb == NTT - 1))
                nc.tensor.matmul(pvB[0 : DH + 1, :], V[:, kb, hB, :],
                                 es[:, 512:1024],
                                 start=(kb == 0), stop=(kb == NTT - 1))

        def normalize(p, qq, pvA, pvB):
            """Evacuate pv fast (frees PSUM), then 1/denom and scale."""
            cs = slice(qq * 512, (qq + 1) * 512)
            for pv, r0 in ((pvA, 0), (pvB, DH)):
                pvr = bcpool.tile([P, 512], f32, tag="pvr", name="pvr")
                nc.vector.tensor_copy(pvr[0 : DH + 1, :], pv[0 : DH + 1, :])
                rec = bcpool.tile([1, 512], f32, tag="rec", name="rec")
                nc.vector.reciprocal(rec[:, :], pvr[DH : DH + 1, :])
                bcs = bcpool.tile([P, 512], f32, tag="bc", name="bc")
                nc.gpsimd.partition_broadcast(bcs[:, :], rec[0:1, :],
                                              channels=P)
                nc.vector.tensor_mul(outT[p][r0 : r0 + DH, cs],
                                     pvr[0:DH, :], bcs[0:DH, :])

        # one-time feature-row broadcasts for the V correction
        nc.gpsimd.partition_broadcast(wsv_bc[:, :], ws_sb[0:1, 2, :],
                                      channels=P)
        if apply_b:
            nc.gpsimd.partition_broadcast(bv_bc[:, :], wb_sb[0:1, 2, :],
                                          channels=P)

        # ---- prefix interleaved with attention window (p0, qq0) ----
        pvA0 = pvpool.tile([P, 512], f32, tag="pv", name="pv")
        pvB0 = pvpool.tile([P, 512], f32, tag="pv", name="pv")
        for qb in range(NQB):
            stats_chunk(qb)
            for u in qk_tile_units(QT[0], wq_sb, 0, 0, qb):
                u()
            for u in qk_tile_units(KT[0], wk_sb, 1, 0, qb):
                u()
            for t in range(qb * 4, qb * 4 + 4):
                v_tile(t)
            att_block(0, 0, pvA0, pvB0, range(qb * 4, qb * 4 + 4), bg=False)
        normalize(0, 0, pvA0, pvB0)

        # ---- remaining attention windows with background work ----
        for p in range(4):
            if p < 3:
                for qb in range(NQB):
                    bgq.extend(qk_tile_units(QT[p + 1], wq_sb, 0, p + 1, qb))
                for qb in range(NQB):
                    bgq.extend(qk_tile_units(KT[p + 1], wk_sb, 1, p + 1, qb))
            for qq in range(1, NQB) if p == 0 else range(NQB):
                pvA = pvpool.tile([P, 512], f32, tag="pv", name="pv")
                pvB = pvpool.tile([P, 512], f32, tag="pv", name="pv")
                att_block(p, qq, pvA, pvB, range(NTT), bg=True)
                normalize(p, qq, pvA, pvB)
                if p == 3:
                    for t in range(qq * 4, qq * 4 + 4):
                        bgq.extend(proj_units(t))
        while bgq:
            bgq.pop(0)()


def _host_prep(x, ln_gamma, ln_beta, w_qkv, w_out, apply_b):
    W3 = np.asarray(w_qkv, dtype=np.float32) * np.asarray(
        ln_gamma, dtype=np.float32)[None, :]

    def chunks(W):  # W [512 feat, 1024 dim] -> lhsT tiles [128, 8, 512]
        wt = W.T.reshape(NDT, P, GI)  # [8, 128, 512]
        return np.ascontiguousarray(
            wt.transpose(1, 0, 2), dtype=np.float16)

    beta = np.asarray(ln_beta, dtype=np.float32)
    in_maps = []
    for b in range(4):
        xtb = np.ascontiguousarray(np.asarray(x[b], np.float32).T,
                                   dtype=np.float16)
        for g in range(2):
            lo, hi = g * GI, (g + 1) * GI
            Wq = W3[lo:hi] * SCALE
            Wk = W3[INNER + lo : INNER + hi]
            Wv = W3[2 * INNER + lo : 2 * INNER + hi]
            m = {
                "xt": xtb,
                "wq": chunks(Wq),
                "wk": chunks(Wk),
                "wv": chunks(Wv),
                "wo": np.ascontiguousarray(
                    np.asarray(w_out, np.float32)[:, lo:hi].T.reshape(
                        4, P, DIM), dtype=np.float16),
                "ws": np.ascontiguousarray(
                    np.stack([Wq.sum(1), Wk.sum(1), Wv.sum(1)])[None],
                    dtype=np.float16),
                "wsc": np.ascontiguousarray(
                    np.stack([Wq.sum(1), Wk.sum(1), Wv.sum(1)],
                             axis=1).reshape(4, P, 3).transpose(1, 2, 0),
                    dtype=np.float32),
            }
            if apply_b:
                m["wb"] = np.ascontiguousarray(
                    np.stack([Wq @ beta, Wk @ beta, Wv @ beta])[None],
                    dtype=np.float16)
                m["wbc"] = np.ascontiguousarray(
                    np.stack([Wq @ beta, Wk @ beta],
                             axis=1).reshape(4, P, 2).transpose(1, 2, 0),
                    dtype=np.float32)
            in_maps.append(m)
    return in_maps


def _run(inputs, trace=False):
    ln_gamma = np.asarray(inputs["ln_gamma"], dtype=np.float32)
    ln_beta = np.asarray(inputs["ln_beta"], dtype=np.float32)
    apply_b = bool((ln_beta != 0.0).any())
    key = ("nc", apply_b)
    if key not in _CACHE:
        _CACHE[key] = build_nc(apply_b=apply_b)
    nc = _CACHE[key]
    in_maps = _host_prep(inputs["x"], ln_gamma, ln_beta,
                         inputs["w_qkv"], inputs["w_out"], apply_b)
    res = run_bass_kernel_spmd(nc, in_maps, list(range(N_CORES)), trace=trace)
    b_out = np.asarray(inputs["b_out"], dtype=np.float32)
    out = np.empty((4, N_TOK, DIM), dtype=np.float32)
    for b in range(4):
        out[b] = (res.results[2 * b]["out"].astype(np.float32)
                  + res.results[2 * b + 1]["out"].astype(np.float32)
                  + b_out[None, :])
    return out, res


def kernel(**inputs):
    out, _ = _run(inputs, trace=False)
    return out


def kernel_profiled(**inputs):
    out, res = _run(inputs, trace=True)
    return out, res
